# revision 6
# baseline (speedup 1.0000x reference)
"""Trainium2 Bass kernel for DisorderedCausalSelfAttention.

Full shapes: x[4,2048,1024], W_attn[1024,3072], W_proj[1024,1024],
bQ/bK[16,64].  Sharding: 8 cores = 4 batches x 2 head-groups (8 heads
each).  Each core computes qkv projection for its heads, causal
attention, and a partial output projection; the host sums the two
partials per batch and adds b_proj.

Per-core dataflow (T=2048, 8 local heads = 4 head-pairs):
  - xT [1024,2048] (host-transposed) -> SBUF as 8 [128,2048] chunks.
  - v  = xT.T @ Wv  in natural [t,d] layout, stored interleaved with a
    ones column per head ([128, 8*65]) so the PV matmul also produces
    the softmax denominator (row 64 of the [65,512] psum).
  - q/k produced transposed [d,t], head-pairs stacked on 128 partitions,
    fp32r matmuls; combined bias (b_attn + bQ/bK) added during the
    PSUM->SBUF copy as a per-partition tensor_scalar add.
  - scores S^T[k,q]: two heads run concurrently via PE row-tiling
    (p=64 each, tile_position auto from base_partition 0/64).
  - exp on ACT with scale=1/8 folded in, reading [128,1024] PSUM (two
    k-chunks per op) -> bf16 Et in SBUF; no max-subtraction (scores/8
    are bounded ~[-3,6] for this distribution).
  - causal mask: bf16 0/1 mask multiply on the 4 diagonal-straddling
    blocks per (head, q-strip) only.
  - PV: yU^T[65,512] = vaug^T @ Et accumulated over k-chunks in PSUM.
  - normalize: recip of denom row, gpsimd partition_broadcast, DVE mul
    -> yT bf16 [128,2048] per head-pair.
  - proj: out[t,1024] partial = yT.T @ Wp(bf16), accumulated over the
    4 head-pair chunks in PSUM, DMA'd out as fp32.
"""

import numpy as np
import ml_dtypes

import concourse.bass as bass
import concourse.bacc as bacc
import concourse.mybir as mybir
import concourse.tile as tile
from concourse.bass_utils import run_bass_kernel_spmd

B, T, C, NH, D = 4, 2048, 1024, 16, 64
NCORES = 8
HPC = 8          # heads per core
NPAIR = 4        # head pairs per core
TS = 512         # q-strip size
KC = 128         # k-chunk size

f32 = mybir.dt.float32
f32r = mybir.dt.float32r
bf16 = mybir.dt.bfloat16
AF = mybir.ActivationFunctionType


def build_nc(t=T, bass_cls=bacc.Bacc):
    ns = t // TS           # q strips
    nkc = t // KC          # k chunks
    ntc = t // KC          # t chunks (same granularity)
    nc = bass_cls()

    xt_d = nc.declare_dram_parameter("xt", [8, 128, t], f32r, isOutput=False)
    wqk_d = nc.declare_dram_parameter("wqk", [8, 1024, 128], f32r, isOutput=False)
    wv_d = nc.declare_dram_parameter("wv", [8, 128, 512], f32r, isOutput=False)
    wp_d = nc.declare_dram_parameter("wp", [4, 128, 1024], bf16, isOutput=False)
    bqk_d = nc.declare_dram_parameter("bqk", [8, 128, 1], f32, isOutput=False)
    bv_d = nc.declare_dram_parameter("bv", [1, 512], f32r, isOutput=False)
    ones_d = nc.declare_dram_parameter("ones", [1, 128], f32r, isOutput=False)
    msk_d = nc.declare_dram_parameter("masks", [4, 128, 512], bf16, isOutput=False)
    out_d = nc.declare_dram_parameter("outp", [t, 1024], f32, isOutput=True)

    with tile.TileContext(nc) as tc:
        with (
            tc.tile_pool(name="persist", bufs=1) as P,
            tc.tile_pool(name="wrot", bufs=16) as WR,
            tc.tile_pool(name="qkt", bufs=4) as QK,
            tc.tile_pool(name="et", bufs=4) as ET,
            tc.tile_pool(name="stage", bufs=4) as SG,
            tc.tile_pool(name="rsb", bufs=2) as RS,
            tc.tile_pool(name="ps_mm", bufs=2, space="PSUM") as PSMM,
            tc.tile_pool(name="ps_st", bufs=2, space="PSUM") as PSST,
            tc.tile_pool(name="ps_yu", bufs=2, space="PSUM") as PSYU,
        ):
            # ---- constant / weight loads -------------------------------
            xt_sb = []
            for p in range(8):
                xtile = P.tile([128, t], f32r, tag=f"xt{p}", name=f"xt{p}")
                nc.sync.dma_start(out=xtile, in_=xt_d[p])
                xt_sb.append(xtile)
            wv_sb = []
            for p in range(8):
                w = P.tile([128, 512], f32r, tag=f"wv{p}", name=f"wv{p}")
                nc.sync.dma_start(out=w, in_=wv_d[p])
                wv_sb.append(w)
            bv_sb = P.tile([1, 512], f32r, tag="bv")
            nc.sync.dma_start(out=bv_sb, in_=bv_d[:, :])
            bqk_sb = P.tile([128, 8], f32, tag="bqk")
            for mc in range(8):
                nc.sync.dma_start(out=bqk_sb[:, mc : mc + 1], in_=bqk_d[mc])
            msk_sb = P.tile([128, 4 * 512], bf16, tag="masks")
            for m in range(4):
                nc.sync.dma_start(out=msk_sb[:, m * 512 : (m + 1) * 512], in_=msk_d[m])
            wp_sb = []
            for j in range(4):
                w = P.tile([128, 1024], bf16, tag=f"wp{j}", name=f"wp{j}")
                nc.sync.dma_start(out=w, in_=wp_d[j])
                wp_sb.append(w)
            ones_sb = P.tile([1, 128], f32r, tag="ones")
            nc.sync.dma_start(out=ones_sb, in_=ones_d[:, :])

            # ---- v projection (natural layout, interleaved ones col) ---
            vaug = [None] * ntc

            def emit_v(tc_i):
                ps = PSMM.tile([128, 512], f32, tag="mm", name="psmm")
                for p in range(8):
                    nc.tensor.matmul(
                        out=ps,
                        lhsT=xt_sb[p][:, tc_i * 128 : (tc_i + 1) * 128],
                        rhs=wv_sb[p],
                        start=(p == 0),
                        stop=False,
                    )
                nc.tensor.matmul(
                    out=ps,
                    lhsT=ones_sb,
                    rhs=bv_sb,
                    start=False,
                    stop=True,
                )
                vt = P.tile([128, 8 * 65], bf16, tag=f"vaug{tc_i}", name=f"vaug{tc_i}")
                v3 = vt.rearrange("p (h c) -> p h c", c=65)
                nc.vector.tensor_copy(
                    out=v3[:, :, 0:64], in_=ps.rearrange("p (h c) -> p h c", c=64)
                )
                nc.vector.memset(v3[:, :, 64:65], 1.0)
                vaug[tc_i] = vt

            # ---- q/k projection for one pair-chunk (128 c' cols) -------
            qkt_tiles = [None] * 8

            def emit_qk(mc):
                wts = []
                for p in range(8):
                    w = WR.tile([128, 128], f32r, tag="wqk", name="wqkt")
                    nc.sync.dma_start(out=w, in_=wqk_d[mc, p * 128 : (p + 1) * 128, :])
                    wts.append(w)
                qk = QK.tile([128, t], f32r, tag="qkt", name="qk")
                for s in range(ns):
                    ps = PSMM.tile([128, 512], f32, tag="mm", name="psmm")
                    for p in range(8):
                        nc.tensor.matmul(
                            out=ps,
                            lhsT=wts[p],
                            rhs=xt_sb[p][:, s * 512 : (s + 1) * 512],
                            start=(p == 0),
                            stop=(p == 7),
                        )
                    nc.vector.tensor_scalar_add(
                        out=qk[:, s * 512 : (s + 1) * 512],
                        in0=ps,
                        scalar1=bqk_sb[:, mc : mc + 1],
                    )
                qkt_tiles[mc] = qk

            # ---- attention for one (pair, strip) -----------------------
            yt_tiles = [None] * NPAIR

            def emit_att_strip(j, s):
                qt = qkt_tiles[2 * j]
                kt = qkt_tiles[2 * j + 1]
                nkc_s = 4 * s + 4  # k-chunks for this strip (causal)
                qs = slice(s * 512, (s + 1) * 512)
                yu = [
                    PSYU.tile([128, 512], f32, tag="yu", name="yuA"),
                    PSYU.tile([128, 512], f32, tag="yu", name="yuB"),
                ]
                for kc0 in range(0, nkc_s, 2):
                    st = [
                        PSST.tile([128, 1024], f32, tag="st", name="stA"),
                        PSST.tile([128, 1024], f32, tag="st", name="stB"),
                    ]
                    for kc in (kc0, kc0 + 1):
                        half = slice((kc % 2) * 512, (kc % 2) * 512 + 512)
                        for hh in range(2):
                            rows = slice(hh * 64, hh * 64 + 64)
                            nc.tensor.matmul(
                                out=st[hh][:, half],
                                lhsT=kt[rows, kc * 128 : (kc + 1) * 128],
                                rhs=qt[rows, qs],
                                start=True,
                                stop=True,
                            )
                    et = []
                    for hh in range(2):
                        e = ET.tile([128, 1024], bf16, tag="et", name="et")
                        nc.scalar.activation(out=e, in_=st[hh], func=AF.Exp, scale=0.125)
                        et.append(e)
                    for kc in (kc0, kc0 + 1):
                        half = slice((kc % 2) * 512, (kc % 2) * 512 + 512)
                        if kc >= 4 * s:
                            m = kc - 4 * s
                            msl = slice(m * 512, (m + 1) * 512)
                            for hh in range(2):
                                nc.vector.tensor_mul(
                                    out=et[hh][:, half],
                                    in0=et[hh][:, half],
                                    in1=msk_sb[:, msl],
                                )
                    for hh in range(2):
                        h = 2 * j + hh
                        for kc in (kc0, kc0 + 1):
                            half = slice((kc % 2) * 512, (kc % 2) * 512 + 512)
                            nc.tensor.matmul(
                                out=yu[hh][0:65, :],
                                lhsT=vaug[kc][:, h * 65 : h * 65 + 65],
                                rhs=et[hh][:, half],
                                start=(kc == 0),
                                stop=(kc == nkc_s - 1),
                            )
                # normalize -> yT
                for hh in range(2):
                    r_row = RS.tile([1, 512], f32, tag="rrow", name="rrow")
                    nc.vector.reciprocal(out=r_row, in_=yu[hh][64:65, :])
                    rs = RS.tile([64, 512], f32, tag="rsb", name="rsb")
                    nc.gpsimd.partition_broadcast(out_ap=rs, in_ap=r_row)
                    nc.vector.tensor_mul(
                        out=yt_tiles[j][hh * 64 : hh * 64 + 64, qs],
                        in0=yu[hh][0:64, :],
                        in1=rs,
                    )

            # ---- emission schedule -------------------------------------
            for tc_i in range(min(8, ntc)):
                emit_v(tc_i)
            emit_qk(0)
            emit_qk(1)
            for j in range(NPAIR):
                yt_tiles[j] = P.tile([128, t], bf16, tag=f"yt{j}", name=f"yt{j}")
                for s in range(ns):
                    emit_att_strip(j, s)
                    if j == 0 and s == 0:
                        for tc_i in range(8, ntc):
                            emit_v(tc_i)
                if 2 * j + 2 < 8:
                    emit_qk(2 * j + 2)
                    emit_qk(2 * j + 3)

            # ---- output projection -------------------------------------
            for tc_i in range(ntc):
                for nsl in range(2):
                    ps = PSMM.tile([128, 512], f32, tag="mm", name="psmm")
                    for j in range(4):
                        nc.tensor.matmul(
                            out=ps,
                            lhsT=yt_tiles[j][:, tc_i * 128 : (tc_i + 1) * 128],
                            rhs=wp_sb[j][:, nsl * 512 : (nsl + 1) * 512],
                            start=(j == 0),
                            stop=(j == 3),
                        )
                    stg = SG.tile([128, 512], f32, tag="stage", name="stage")
                    nc.vector.tensor_copy(out=stg, in_=ps)
                    nc.sync.dma_start(
                        out=out_d[tc_i * 128 : (tc_i + 1) * 128, nsl * 512 : (nsl + 1) * 512],
                        in_=stg,
                    )
    return nc


def make_masks():
    r = np.arange(128)[:, None]
    c = np.arange(512)[None, :]
    return np.stack(
        [(c >= 128 * m + r) for m in range(4)]
    ).astype(ml_dtypes.bfloat16)


def prep_core_inputs(core, x, W_attn, b_attn, bQ, bK, W_proj, masks, t=T):
    b, hg = core // 2, core % 2
    h0 = hg * HPC
    c0 = h0 * D                    # column offset within each of q/k/v blocks
    xt = np.ascontiguousarray(x[b].T).reshape(8, 128, t)
    wq = W_attn[:, c0 : c0 + 512]
    wk = W_attn[:, C + c0 : C + c0 + 512]
    wqk = np.empty((8, 1024, 128), np.float32)
    for j in range(4):
        wqk[2 * j] = wq[:, j * 128 : (j + 1) * 128]
        wqk[2 * j + 1] = wk[:, j * 128 : (j + 1) * 128]
    bq = b_attn[c0 : c0 + 512] + bQ[h0 : h0 + HPC].reshape(512)
    bk = b_attn[C + c0 : C + c0 + 512] + bK[h0 : h0 + HPC].reshape(512)
    bqk = np.empty((8, 128, 1), np.float32)
    for j in range(4):
        bqk[2 * j] = bq[j * 128 : (j + 1) * 128, None]
        bqk[2 * j + 1] = bk[j * 128 : (j + 1) * 128, None]
    wv = np.ascontiguousarray(
        W_attn[:, 2 * C + c0 : 2 * C + c0 + 512]
    ).reshape(8, 128, 512)
    bv = np.ascontiguousarray(b_attn[2 * C + c0 : 2 * C + c0 + 512]).reshape(1, 512)
    wp = np.ascontiguousarray(W_proj[c0 : c0 + 512, :]).reshape(4, 128, 1024).astype(
        ml_dtypes.bfloat16
    )
    return {
        "xt": xt,
        "wqk": wqk,
        "wv": wv,
        "wp": wp,
        "bqk": bqk,
        "bv": bv,
        "ones": np.ones((1, 128), np.float32),
        "masks": masks,
    }


_NC_CACHE = {}


def kernel(x, W_attn, b_attn, W_proj, b_proj, bQ, bK, _return_raw=False):
    x = np.asarray(x, np.float32)
    W_attn = np.asarray(W_attn, np.float32)
    b_attn = np.asarray(b_attn, np.float32)
    W_proj = np.asarray(W_proj, np.float32)
    b_proj = np.asarray(b_proj, np.float32)
    bQ = np.asarray(bQ, np.float32)
    bK = np.asarray(bK, np.float32)

    if "nc" not in _NC_CACHE:
        nc = build_nc()
        nc.finalize()
        _NC_CACHE["nc"] = nc
    nc = _NC_CACHE["nc"]

    masks = make_masks()
    in_maps = [
        prep_core_inputs(c, x, W_attn, b_attn, bQ, bK, W_proj, masks)
        for c in range(NCORES)
    ]
    res = run_bass_kernel_spmd(nc, in_maps, list(range(NCORES)))
    out = np.empty((B, T, C), np.float32)
    for b in range(B):
        out[b] = res.results[2 * b]["outp"] + res.results[2 * b + 1]["outp"] + b_proj
    if _return_raw:
        return out, res
    return (out, bQ)


# revision 7
# speedup vs baseline: 1.1911x; 1.1911x over previous
"""Trainium2 Bass kernel for DisorderedCausalSelfAttention.

Full shapes: x[4,2048,1024], W_attn[1024,3072], W_proj[1024,1024],
bQ/bK[16,64].  Sharding: 8 cores = 4 batches x 2 head-groups (8 heads
each).  Each core computes qkv projection for its heads, causal
attention, and a partial output projection; the host sums the two
partials per batch and adds b_proj.

Per-core dataflow (T=2048, 8 local heads = 4 head-pairs):
  - xT [1024,2048] (host-transposed) -> SBUF as 8 [128,2048] chunks.
  - v  = xT.T @ Wv  in natural [t,d] layout, stored interleaved with a
    ones column per head ([128, 8*65]) so the PV matmul also produces
    the softmax denominator (row 64 of the [65,512] psum).
  - q/k produced transposed [d,t], head-pairs stacked on 128 partitions,
    fp32r matmuls; combined bias (b_attn + bQ/bK) added during the
    PSUM->SBUF copy as a per-partition tensor_scalar add.
  - scores S^T[k,q]: two heads run concurrently via PE row-tiling
    (p=64 each, tile_position auto from base_partition 0/64).
  - exp on ACT with scale=1/8 folded in, reading [128,1024] PSUM (two
    k-chunks per op) -> bf16 Et in SBUF; no max-subtraction (scores/8
    are bounded ~[-3,6] for this distribution).
  - causal mask: bf16 0/1 mask multiply on the 4 diagonal-straddling
    blocks per (head, q-strip) only.
  - PV: yU^T[65,512] = vaug^T @ Et accumulated over k-chunks in PSUM.
  - normalize: recip of denom row, gpsimd partition_broadcast, DVE mul
    -> yT bf16 [128,2048] per head-pair.
  - proj: out[t,1024] partial = yT.T @ Wp(bf16), accumulated over the
    4 head-pair chunks in PSUM, DMA'd out as fp32.
"""

import numpy as np
import ml_dtypes

import concourse.bass as bass
import concourse.bacc as bacc
import concourse.mybir as mybir
import concourse.tile as tile
from concourse.bass_utils import run_bass_kernel_spmd

B, T, C, NH, D = 4, 2048, 1024, 16, 64
NCORES = 8
HPC = 8          # heads per core
NPAIR = 4        # head pairs per core
TS = 512         # q-strip size
KC = 128         # k-chunk size

f32 = mybir.dt.float32
f32r = mybir.dt.float32r
bf16 = mybir.dt.bfloat16
AF = mybir.ActivationFunctionType


def build_nc(t=T, bass_cls=bacc.Bacc):
    ns = t // TS           # q strips
    nkc = t // KC          # k chunks
    ntc = t // KC          # t chunks (same granularity)
    nc = bass_cls()

    xt_d = nc.declare_dram_parameter("xt", [8, 128, t], bf16, isOutput=False)
    wqk_d = nc.declare_dram_parameter("wqk", [8, 1024, 128], bf16, isOutput=False)
    wv_d = nc.declare_dram_parameter("wv", [8, 128, 512], bf16, isOutput=False)
    wp_d = nc.declare_dram_parameter("wp", [4, 128, 1024], bf16, isOutput=False)
    bqk_d = nc.declare_dram_parameter("bqk", [8, 128, 1], f32, isOutput=False)
    bv_d = nc.declare_dram_parameter("bv", [1, 512], bf16, isOutput=False)
    ones_d = nc.declare_dram_parameter("ones", [1, 128], bf16, isOutput=False)
    msk_d = nc.declare_dram_parameter("masks", [4, 128, 512], bf16, isOutput=False)
    out_d = nc.declare_dram_parameter("outp", [t, 1024], f32, isOutput=True)

    with tile.TileContext(nc) as tc:
        with (
            tc.tile_pool(name="persist", bufs=1) as P,
            tc.tile_pool(name="wrot", bufs=16) as WR,
            tc.tile_pool(name="qkt", bufs=4) as QK,
            tc.tile_pool(name="et", bufs=4) as ET,
            tc.tile_pool(name="stage", bufs=4) as SG,
            tc.tile_pool(name="rsb", bufs=2) as RS,
            tc.tile_pool(name="ps_mm", bufs=2, space="PSUM") as PSMM,
            tc.tile_pool(name="ps_st", bufs=2, space="PSUM") as PSST,
            tc.tile_pool(name="ps_yu", bufs=2, space="PSUM") as PSYU,
        ):
            # ---- constant / weight loads -------------------------------
            xt_sb = []
            for p in range(8):
                xtile = P.tile([128, t], bf16, tag=f"xt{p}", name=f"xt{p}")
                nc.sync.dma_start(out=xtile, in_=xt_d[p])
                xt_sb.append(xtile)
            wv_sb = []
            for p in range(8):
                w = P.tile([128, 512], bf16, tag=f"wv{p}", name=f"wv{p}")
                nc.sync.dma_start(out=w, in_=wv_d[p])
                wv_sb.append(w)
            bv_sb = P.tile([1, 512], bf16, tag="bv")
            nc.sync.dma_start(out=bv_sb, in_=bv_d[:, :])
            bqk_sb = P.tile([128, 8], f32, tag="bqk")
            for mc in range(8):
                nc.sync.dma_start(out=bqk_sb[:, mc : mc + 1], in_=bqk_d[mc])
            msk_sb = P.tile([128, 4 * 512], bf16, tag="masks")
            for m in range(4):
                nc.sync.dma_start(out=msk_sb[:, m * 512 : (m + 1) * 512], in_=msk_d[m])
            wp_sb = []
            for j in range(4):
                w = P.tile([128, 1024], bf16, tag=f"wp{j}", name=f"wp{j}")
                nc.sync.dma_start(out=w, in_=wp_d[j])
                wp_sb.append(w)
            ones_sb = P.tile([1, 128], bf16, tag="ones")
            nc.sync.dma_start(out=ones_sb, in_=ones_d[:, :])

            # ---- v projection (natural layout, interleaved ones col) ---
            vaug = [None] * ntc

            def emit_v(tc_i):
                ps = PSMM.tile([128, 512], f32, tag="mm", name="psmm")
                for p in range(8):
                    nc.tensor.matmul(
                        out=ps,
                        lhsT=xt_sb[p][:, tc_i * 128 : (tc_i + 1) * 128],
                        rhs=wv_sb[p],
                        start=(p == 0),
                        stop=False,
                    )
                nc.tensor.matmul(
                    out=ps,
                    lhsT=ones_sb,
                    rhs=bv_sb,
                    start=False,
                    stop=True,
                )
                vt = P.tile([128, 8 * 65], bf16, tag=f"vaug{tc_i}", name=f"vaug{tc_i}")
                v3 = vt.rearrange("p (h c) -> p h c", c=65)
                nc.vector.tensor_copy(
                    out=v3[:, :, 0:64], in_=ps.rearrange("p (h c) -> p h c", c=64)
                )
                nc.vector.memset(v3[:, :, 64:65], 1.0)
                vaug[tc_i] = vt

            # ---- q/k projection for one pair-chunk (128 c' cols) -------
            qkt_tiles = [None] * 8

            def emit_qk(mc):
                wts = []
                for p in range(8):
                    w = WR.tile([128, 128], bf16, tag="wqk", name="wqkt")
                    nc.sync.dma_start(out=w, in_=wqk_d[mc, p * 128 : (p + 1) * 128, :])
                    wts.append(w)
                qk = QK.tile([128, t], f32r, tag="qkt", name="qk")
                for s in range(ns):
                    ps = PSMM.tile([128, 512], f32, tag="mm", name="psmm")
                    for p in range(8):
                        nc.tensor.matmul(
                            out=ps,
                            lhsT=wts[p],
                            rhs=xt_sb[p][:, s * 512 : (s + 1) * 512],
                            start=(p == 0),
                            stop=(p == 7),
                        )
                    nc.vector.tensor_scalar_add(
                        out=qk[:, s * 512 : (s + 1) * 512],
                        in0=ps,
                        scalar1=bqk_sb[:, mc : mc + 1],
                    )
                qkt_tiles[mc] = qk

            # ---- attention for one (pair, strip) -----------------------
            yt_tiles = [None] * NPAIR

            def emit_att_strip(j, s):
                qt = qkt_tiles[2 * j]
                kt = qkt_tiles[2 * j + 1]
                nkc_s = 4 * s + 4  # k-chunks for this strip (causal)
                qs = slice(s * 512, (s + 1) * 512)
                yu = [
                    PSYU.tile([128, 512], f32, tag="yu", name="yuA"),
                    PSYU.tile([128, 512], f32, tag="yu", name="yuB"),
                ]
                for kc0 in range(0, nkc_s, 2):
                    st = [
                        PSST.tile([128, 1024], f32, tag="st", name="stA"),
                        PSST.tile([128, 1024], f32, tag="st", name="stB"),
                    ]
                    for kc in (kc0, kc0 + 1):
                        half = slice((kc % 2) * 512, (kc % 2) * 512 + 512)
                        for hh in range(2):
                            rows = slice(hh * 64, hh * 64 + 64)
                            nc.tensor.matmul(
                                out=st[hh][:, half],
                                lhsT=kt[rows, kc * 128 : (kc + 1) * 128],
                                rhs=qt[rows, qs],
                                start=True,
                                stop=True,
                            )
                    et = []
                    for hh in range(2):
                        e = ET.tile([128, 1024], bf16, tag="et", name="et")
                        nc.scalar.activation(out=e, in_=st[hh], func=AF.Exp, scale=0.125)
                        et.append(e)
                    for kc in (kc0, kc0 + 1):
                        half = slice((kc % 2) * 512, (kc % 2) * 512 + 512)
                        if kc >= 4 * s:
                            m = kc - 4 * s
                            msl = slice(m * 512, (m + 1) * 512)
                            for hh in range(2):
                                nc.vector.tensor_mul(
                                    out=et[hh][:, half],
                                    in0=et[hh][:, half],
                                    in1=msk_sb[:, msl],
                                )
                    for hh in range(2):
                        h = 2 * j + hh
                        for kc in (kc0, kc0 + 1):
                            half = slice((kc % 2) * 512, (kc % 2) * 512 + 512)
                            nc.tensor.matmul(
                                out=yu[hh][0:65, :],
                                lhsT=vaug[kc][:, h * 65 : h * 65 + 65],
                                rhs=et[hh][:, half],
                                start=(kc == 0),
                                stop=(kc == nkc_s - 1),
                            )
                # normalize -> yT
                for hh in range(2):
                    r_row = RS.tile([1, 512], f32, tag="rrow", name="rrow")
                    nc.vector.reciprocal(out=r_row, in_=yu[hh][64:65, :])
                    rs = RS.tile([64, 512], f32, tag="rsb", name="rsb")
                    nc.gpsimd.partition_broadcast(out_ap=rs, in_ap=r_row)
                    nc.vector.tensor_mul(
                        out=yt_tiles[j][hh * 64 : hh * 64 + 64, qs],
                        in0=yu[hh][0:64, :],
                        in1=rs,
                    )

            # ---- emission schedule -------------------------------------
            for tc_i in range(min(8, ntc)):
                emit_v(tc_i)
            emit_qk(0)
            emit_qk(1)
            for j in range(NPAIR):
                yt_tiles[j] = P.tile([128, t], bf16, tag=f"yt{j}", name=f"yt{j}")
                for s in range(ns):
                    emit_att_strip(j, s)
                    if j == 0 and s == 0:
                        for tc_i in range(8, ntc):
                            emit_v(tc_i)
                if 2 * j + 2 < 8:
                    emit_qk(2 * j + 2)
                    emit_qk(2 * j + 3)

            # ---- output projection -------------------------------------
            for tc_i in range(ntc):
                for nsl in range(2):
                    ps = PSMM.tile([128, 512], f32, tag="mm", name="psmm")
                    for j in range(4):
                        nc.tensor.matmul(
                            out=ps,
                            lhsT=yt_tiles[j][:, tc_i * 128 : (tc_i + 1) * 128],
                            rhs=wp_sb[j][:, nsl * 512 : (nsl + 1) * 512],
                            start=(j == 0),
                            stop=(j == 3),
                        )
                    stg = SG.tile([128, 512], f32, tag="stage", name="stage")
                    nc.vector.tensor_copy(out=stg, in_=ps)
                    nc.sync.dma_start(
                        out=out_d[tc_i * 128 : (tc_i + 1) * 128, nsl * 512 : (nsl + 1) * 512],
                        in_=stg,
                    )
    return nc


def make_masks():
    r = np.arange(128)[:, None]
    c = np.arange(512)[None, :]
    return np.stack(
        [(c >= 128 * m + r) for m in range(4)]
    ).astype(ml_dtypes.bfloat16)


def prep_core_inputs(core, x, W_attn, b_attn, bQ, bK, W_proj, masks, t=T):
    b, hg = core // 2, core % 2
    h0 = hg * HPC
    c0 = h0 * D                    # column offset within each of q/k/v blocks
    xt = np.ascontiguousarray(x[b].T).astype(ml_dtypes.bfloat16).reshape(8, 128, t)
    wq = W_attn[:, c0 : c0 + 512]
    wk = W_attn[:, C + c0 : C + c0 + 512]
    wqk = np.empty((8, 1024, 128), ml_dtypes.bfloat16)
    for j in range(4):
        wqk[2 * j] = wq[:, j * 128 : (j + 1) * 128]
        wqk[2 * j + 1] = wk[:, j * 128 : (j + 1) * 128]
    bq = b_attn[c0 : c0 + 512] + bQ[h0 : h0 + HPC].reshape(512)
    bk = b_attn[C + c0 : C + c0 + 512] + bK[h0 : h0 + HPC].reshape(512)
    bqk = np.empty((8, 128, 1), np.float32)
    for j in range(4):
        bqk[2 * j] = bq[j * 128 : (j + 1) * 128, None]
        bqk[2 * j + 1] = bk[j * 128 : (j + 1) * 128, None]
    wv = np.ascontiguousarray(
        W_attn[:, 2 * C + c0 : 2 * C + c0 + 512]
    ).astype(ml_dtypes.bfloat16).reshape(8, 128, 512)
    bv = np.ascontiguousarray(b_attn[2 * C + c0 : 2 * C + c0 + 512]).astype(
        ml_dtypes.bfloat16
    ).reshape(1, 512)
    wp = np.ascontiguousarray(W_proj[c0 : c0 + 512, :]).reshape(4, 128, 1024).astype(
        ml_dtypes.bfloat16
    )
    return {
        "xt": xt,
        "wqk": wqk,
        "wv": wv,
        "wp": wp,
        "bqk": bqk,
        "bv": bv,
        "ones": np.ones((1, 128), ml_dtypes.bfloat16),
        "masks": masks,
    }


_NC_CACHE = {}


def kernel(x, W_attn, b_attn, W_proj, b_proj, bQ, bK, _return_raw=False):
    x = np.asarray(x, np.float32)
    W_attn = np.asarray(W_attn, np.float32)
    b_attn = np.asarray(b_attn, np.float32)
    W_proj = np.asarray(W_proj, np.float32)
    b_proj = np.asarray(b_proj, np.float32)
    bQ = np.asarray(bQ, np.float32)
    bK = np.asarray(bK, np.float32)

    if "nc" not in _NC_CACHE:
        nc = build_nc()
        nc.finalize()
        _NC_CACHE["nc"] = nc
    nc = _NC_CACHE["nc"]

    masks = make_masks()
    in_maps = [
        prep_core_inputs(c, x, W_attn, b_attn, bQ, bK, W_proj, masks)
        for c in range(NCORES)
    ]
    res = run_bass_kernel_spmd(nc, in_maps, list(range(NCORES)))
    out = np.empty((B, T, C), np.float32)
    for b in range(B):
        out[b] = res.results[2 * b]["outp"] + res.results[2 * b + 1]["outp"] + b_proj
    if _return_raw:
        return out, res
    return (out, bQ)
